# revision 1
# baseline (speedup 1.0000x reference)
"""Trainium2 Bass kernel for the ContinuousThoughtMachine problem.

Strategy: data-parallel over batch B=512 across 8 NeuronCores (64 per core).
T-layout on device: feature dims on partitions, batch on the free dim.

Per core, per tick t:
  1. synapse: preT = Wu.T @ zT + cT           (PE, 256 bf16 matmuls, PSUM acc)
  2. NLM:     a_h  = sum_s pre_s * w1[slot]   (DVE broadcast mul/add chains)
              r_h  = relu(a_h)                (ACT)
              z    = sum_h w2_h * r_h         (DVE)
  3. sync:    z -> DRAM, dma_gather pairs, U = z_i*z_j, S = S*r + U   (DMA/DVE)
  4. outputs: y = S @ W_out, q = S @ W_act    (PE, 32 matmuls)

feats = x @ W_in and cT = feats @ Wl + bias are computed once on device
before the tick loop (Wl streamed through SBUF).
"""
import sys
sys.path.insert(0, '/opt/trn_rl_repo')
import numpy as np
import ml_dtypes

import concourse.bass as bass
import concourse.mybir as mybir
from concourse import bacc
from concourse.bass_utils import run_bass_kernel_spmd
from concourse.tile import TileContext

# problem dims
B, T, D, DIN, M, H, P, DOUT, DACT = 512, 8, 2048, 512, 16, 4, 512, 512, 512
NCORES = 8
BC = B // NCORES          # 64 batch per core
DC = D // 128             # 16 d-chunks
PC = P // 128             # 4 p-chunks
KIN = DIN // 128          # 4 input chunks
NGI = 4 * P               # 2048 gather indices (i_out, j_out, i_act, j_act)

F32 = mybir.dt.float32
BF16 = mybir.dt.bfloat16
I16 = mybir.dt.int16
MULT = mybir.AluOpType.mult
ADD = mybir.AluOpType.add

_cache = {}

import os
DBG_TICKS = int(os.environ.get("DBG_TICKS", "8"))
DBG_NO_GATHER = os.environ.get("DBG_NO_GATHER", "0") == "1"
DBG_CORES = int(os.environ.get("DBG_CORES", str(NCORES)))
DBG_DUMP = os.environ.get("DBG_DUMP", "0") == "1"
ACC_BF16 = os.environ.get("ACC_BF16", "1") == "1"     # NLM accumulator dtype
NLM_GP_H = int(os.environ.get("NLM_GP_H", "1"))       # how many h-lanes run on gpsimd
ZC_GP = os.environ.get("ZC_GP", "1") == "1"           # z-combine: half on gpsimd


def _bcast(ap, n, pos=None):
    """Append (or insert) a stride-0 dim of size n to an AP."""
    dims = list(ap.ap)
    if pos is None:
        dims = dims + [[0, n]]
    else:
        dims = dims[:pos] + [[0, n]] + dims[pos:]
    return bass.AP(ap.tensor, ap.offset, dims)


def _build(with_b1, with_b2, with_r):
    nc = bacc.Bacc("TRN2", target_bir_lowering=False, debug=False)

    # ---------------- DRAM tensors ----------------
    xT_d = nc.dram_tensor("xT", [DIN, BC], BF16, kind="ExternalInput")
    win_d = nc.dram_tensor("win", [DIN, D], BF16, kind="ExternalInput")     # lhsT chunks
    wl_d = nc.dram_tensor("wl", [DC * D, 128], BF16, kind="ExternalInput")  # feats half of W_syn, col-blocked
    wu_d = nc.dram_tensor("wu", [D, D], BF16, kind="ExternalInput")         # z half of W_syn
    cb_d = nc.dram_tensor("cb", [128, DC], F32, kind="ExternalInput")       # b_in@Wl + b_syn, [dp, dc]
    z0_d = nc.dram_tensor("z0t", [128, DC], F32, kind="ExternalInput")
    wn_d = nc.dram_tensor("wn", [128, H * 8 * DC], BF16, kind="ExternalInput")   # w1[M-1-u,h,d] -> [dp,(h,u,dc)]
    w2_d = nc.dram_tensor("w2p", [128, H * DC], BF16, kind="ExternalInput")      # [dp,(h,dc)]
    b1_d = nc.dram_tensor("b1p", [128, H * DC], F32, kind="ExternalInput")
    b2_d = nc.dram_tensor("b2p", [128, DC], F32, kind="ExternalInput")
    rr_d = nc.dram_tensor("rr", [128, 2 * PC], F32, kind="ExternalInput")   # r_out, r_act as [pp, pc]
    gi_d = nc.dram_tensor("gi", [128, NGI // 16], I16, kind="ExternalInput")
    wo_d = nc.dram_tensor("wo", [P, DOUT], BF16, kind="ExternalInput")      # lhsT
    wa_d = nc.dram_tensor("wa", [P, DACT], BF16, kind="ExternalInput")

    zdr_d = nc.dram_tensor("zdr", [D, BC], F32, kind="Internal")            # z round-trip for gather
    y_d = nc.dram_tensor("y", [T, DOUT, BC], F32, kind="ExternalOutput")
    q_d = nc.dram_tensor("q", [T, DACT, BC], F32, kind="ExternalOutput")
    if DBG_DUMP:
        dbg_feats = nc.dram_tensor("dbg_feats", [128, DC * BC], F32, kind="ExternalOutput")
        dbg_ct = nc.dram_tensor("dbg_ct", [128, DC * BC], F32, kind="ExternalOutput")
        dbg_h1 = nc.dram_tensor("dbg_h1", [128, DC * BC], F32, kind="ExternalOutput")
        dbg_zf = nc.dram_tensor("dbg_zf", [128, DC * BC], F32, kind="ExternalOutput")
        dbg_zg = nc.dram_tensor("dbg_zg", [128, (NGI // 128) * BC], F32, kind="ExternalOutput")
        dbg_s = nc.dram_tensor("dbg_s", [128, 2 * PC * BC], F32, kind="ExternalOutput")
        dbg_acc = nc.dram_tensor("dbg_acc", [128, H * DC * BC], F32, kind="ExternalOutput")

    with TileContext(nc) as tc:
        with tc.tile_pool(name="const", bufs=1) as cp, \
             tc.tile_pool(name="wlp", bufs=3) as wlp, \
             tc.tile_pool(name="work", bufs=2) as wp, \
             tc.tile_pool(name="psum", bufs=2, space="PSUM") as pp, \
             tc.tile_pool(name="psum_o", bufs=2, space="PSUM") as ppo:

            # ---------------- resident loads ----------------
            wu_sb = cp.tile([128, DC * D], BF16)        # [dp, (kc, m)] 64KB/part
            for kc in range(DC):
                nc.sync.dma_start(out=wu_sb[:, kc * D:(kc + 1) * D],
                                  in_=wu_d[kc * 128:(kc + 1) * 128, :])
            win_sb = cp.tile([128, KIN * D], BF16)
            for kc in range(KIN):
                nc.sync.dma_start(out=win_sb[:, kc * D:(kc + 1) * D],
                                  in_=win_d[kc * 128:(kc + 1) * 128, :])
            xt_sb = cp.tile([128, KIN * BC], BF16)
            nc.sync.dma_start(out=xt_sb[:],
                              in_=xT_d[:].rearrange("(kc p) b -> p kc b", p=128))
            cb_sb = cp.tile([128, DC], F32)
            nc.sync.dma_start(out=cb_sb[:], in_=cb_d[:])
            z0_sb = cp.tile([128, DC], F32)
            nc.sync.dma_start(out=z0_sb[:], in_=z0_d[:])
            wn_sb = cp.tile([128, H * 8 * DC], BF16)
            nc.sync.dma_start(out=wn_sb[:], in_=wn_d[:])
            w2_sb = cp.tile([128, H * DC], BF16)
            nc.sync.dma_start(out=w2_sb[:], in_=w2_d[:])
            b1_sb = cp.tile([128, H * DC], F32)
            if with_b1:
                nc.sync.dma_start(out=b1_sb[:], in_=b1_d[:])
            b2_sb = cp.tile([128, DC], F32)
            if with_b2:
                nc.sync.dma_start(out=b2_sb[:], in_=b2_d[:])
            rr_sb = cp.tile([128, 2 * PC], F32)
            nc.sync.dma_start(out=rr_sb[:], in_=rr_d[:])
            gi_sb = cp.tile([128, NGI // 16], I16)
            nc.sync.dma_start(out=gi_sb[:], in_=gi_d[:])
            wo_sb = cp.tile([128, PC * DOUT], BF16)
            for kc in range(PC):
                nc.sync.dma_start(out=wo_sb[:, kc * DOUT:(kc + 1) * DOUT],
                                  in_=wo_d[kc * 128:(kc + 1) * 128, :])
            wa_sb = cp.tile([128, PC * DACT], BF16)
            for kc in range(PC):
                nc.sync.dma_start(out=wa_sb[:, kc * DACT:(kc + 1) * DACT],
                                  in_=wa_d[kc * 128:(kc + 1) * 128, :])

            # ---------------- feats + cT (once) ----------------
            # featsT[d, b] = sum_k W_in[k, d] * xT[k, b]
            f_ps = pp.tile([128, DC * BC], F32, tag="syn")
            for mc in range(DC):
                for kc in range(KIN):
                    nc.tensor.matmul(
                        out=f_ps[:, mc * BC:(mc + 1) * BC],
                        lhsT=win_sb[:, kc * D + mc * 128: kc * D + (mc + 1) * 128],
                        rhs=xt_sb[:, kc * BC:(kc + 1) * BC],
                        start=(kc == 0), stop=(kc == KIN - 1))
            featsT = cp.tile([128, DC * BC], BF16)
            nc.vector.tensor_copy(out=featsT[:], in_=f_ps[:])

            # cT[d, b] = sum_k Wl[k, d] * featsT[k, b]  + cb (bias)
            # Wl is column-blocked on host: wl_d[mc*D + kc*128 + p, m] = Wl[kc*128+p, mc*128+m]
            # mc-outer so each PSUM region's accumulation group is contiguous
            # (start=True clears has_written at bank granularity).
            c_ps = pp.tile([128, DC * BC], F32, tag="syn")
            for mc in range(DC):
                wl_t = wlp.tile([128, DC * 128], BF16, tag="wl")
                nc.sync.dma_start(
                    out=wl_t[:],
                    in_=wl_d[mc * D:(mc + 1) * D, :].rearrange("(kc p) m -> p kc m", p=128))
                for kc in range(DC):
                    nc.tensor.matmul(
                        out=c_ps[:, mc * BC:(mc + 1) * BC],
                        lhsT=wl_t[:, kc * 128:(kc + 1) * 128],
                        rhs=featsT[:, kc * BC:(kc + 1) * BC],
                        start=(kc == 0), stop=(kc == DC - 1))
            cT = cp.tile([128, DC * BC], F32)
            c_ps3 = c_ps[:].rearrange("p (dc b) -> p dc b", dc=DC)
            nc.vector.tensor_tensor(out=cT[:].rearrange("p (dc b) -> p dc b", dc=DC),
                                    in0=c_ps3, in1=_bcast(cb_sb[:], BC), op=ADD)

            # ---------------- z0 broadcast ----------------
            zT = cp.tile([128, DC * BC], BF16)       # matmul rhs (bf16)
            nc.vector.tensor_copy(out=zT[:].rearrange("p (dc b) -> p dc b", dc=DC),
                                  in_=_bcast(z0_sb[:], BC))

            # ---------------- history + NLM buffers ----------------
            ACCDT = BF16 if ACC_BF16 else F32
            hist = cp.tile([128, T * DC * BC], BF16)   # slot s-1 at [s-1, dc, b]
            acc = cp.tile([128, DC * BC], ACCDT, tag="acc")
            tmp = cp.tile([128, DC * BC], BF16, tag="tmp")
            acc_g = cp.tile([128, DC * BC], ACCDT, tag="accg")   # gpsimd lane
            tmp_g = cp.tile([128, DC * BC], BF16, tag="tmpg")
            zf_g = cp.tile([128, DC * BC], F32, tag="zfg")
            relu_r = cp.tile([128, H * DC * BC], BF16)
            zf = cp.tile([128, DC * BC], F32)
            zg = cp.tile([128, (NGI // 128) * BC], F32)   # gathered rows [pp, (16 c, b)]
            u_t = cp.tile([128, 2 * PC * BC], F32)        # U_out, U_act
            s_t = cp.tile([128, 2 * PC * BC], F32)        # S_out, S_act
            s_bf = cp.tile([128, 2 * PC * BC], BF16)
            nc.vector.memset(s_t[:], 0.0)

            wn3 = wn_sb[:].rearrange("p (h u dc) -> p (h u) dc", h=H, u=8)
            w23 = w2_sb[:].rearrange("p (h dc) -> p h dc", h=H)
            b13 = b1_sb[:].rearrange("p (h dc) -> p h dc", h=H)

            # ---------------- tick loop ----------------
            for t in range(1, DBG_TICKS + 1):
                # --- synapse: preT = Wu.T @ zT (+cT) -> hist slot t-1 ---
                syn_ps = pp.tile([128, DC * BC], F32, tag="syn")
                for mc in range(DC):
                    for kc in range(DC):
                        nc.tensor.matmul(
                            out=syn_ps[:, mc * BC:(mc + 1) * BC],
                            lhsT=wu_sb[:, kc * D + mc * 128: kc * D + (mc + 1) * 128],
                            rhs=zT[:, kc * BC:(kc + 1) * BC],
                            start=(kc == 0), stop=(kc == DC - 1))
                hslot = hist[:, (t - 1) * DC * BC: t * DC * BC]
                nc.vector.tensor_tensor(out=hslot, in0=syn_ps[:], in1=cT[:], op=ADD)

                # --- NLM: h-lanes split DVE / gpsimd ---
                def nlm_lane(eng, h, a_t, m_t):
                    for s in range(1, t + 1):
                        u = t - s
                        hs3 = hist[:, (s - 1) * DC * BC: s * DC * BC].rearrange(
                            "p (dc b) -> p dc b", dc=DC)
                        w_in1 = _bcast(wn3[:, h * 8 + u, :], BC)
                        if s == 1:
                            eng.tensor_tensor(
                                out=a_t[:].rearrange("p (dc b) -> p dc b", dc=DC),
                                in0=hs3, in1=w_in1, op=MULT)
                            if with_b1:
                                eng.tensor_tensor(
                                    out=a_t[:].rearrange("p (dc b) -> p dc b", dc=DC),
                                    in0=a_t[:].rearrange("p (dc b) -> p dc b", dc=DC),
                                    in1=_bcast(b13[:, h, :], BC), op=ADD)
                        else:
                            eng.tensor_tensor(
                                out=m_t[:].rearrange("p (dc b) -> p dc b", dc=DC),
                                in0=hs3, in1=w_in1, op=MULT)
                            eng.tensor_tensor(out=a_t[:], in0=a_t[:], in1=m_t[:], op=ADD)
                    nc.scalar.activation(
                        out=relu_r[:, h * DC * BC:(h + 1) * DC * BC],
                        in_=a_t[:], func=mybir.ActivationFunctionType.Relu)

                for h in range(H):
                    if h >= H - NLM_GP_H:
                        nlm_lane(nc.gpsimd, h, acc_g, tmp_g)
                    else:
                        nlm_lane(nc.vector, h, acc, tmp)

                # z = sum_h w2_h * r_h (+ b2): h 0,1 on DVE -> zf; h 2,3 on gpsimd -> zf_g
                def zc_mul(eng, h, out_t):
                    r3 = relu_r[:, h * DC * BC:(h + 1) * DC * BC].rearrange(
                        "p (dc b) -> p dc b", dc=DC)
                    eng.tensor_tensor(out=out_t, in0=r3,
                                      in1=_bcast(w23[:, h, :], BC), op=MULT)
                zc_mul(nc.vector, 0, zf[:].rearrange("p (dc b) -> p dc b", dc=DC))
                zc_mul(nc.vector, 1, tmp[:].rearrange("p (dc b) -> p dc b", dc=DC))
                nc.vector.tensor_tensor(out=zf[:], in0=zf[:], in1=tmp[:], op=ADD)
                g_eng = nc.gpsimd if ZC_GP else nc.vector
                zc_mul(g_eng, 2, zf_g[:].rearrange("p (dc b) -> p dc b", dc=DC))
                zc_mul(g_eng, 3, tmp_g[:].rearrange("p (dc b) -> p dc b", dc=DC))
                g_eng.tensor_tensor(out=zf_g[:], in0=zf_g[:], in1=tmp_g[:], op=ADD)
                nc.vector.tensor_tensor(out=zf[:], in0=zf[:], in1=zf_g[:], op=ADD)
                if with_b2:
                    nc.vector.tensor_tensor(
                        out=zf[:].rearrange("p (dc b) -> p dc b", dc=DC),
                        in0=zf[:].rearrange("p (dc b) -> p dc b", dc=DC),
                        in1=_bcast(b2_sb[:], BC), op=ADD)
                nc.vector.tensor_copy(out=zT[:], in_=zf[:])   # bf16 for next synapse

                # --- sync path: z -> DRAM -> gather -> U -> S -> y/q ---
                if DBG_NO_GATHER:
                    nc.vector.tensor_copy(out=zg[:], in_=_bcast(zf[:, 0:1], NGI // 128 * BC)[:, 0, :])
                else:
                    nc.sync.dma_start(
                        out=zdr_d[:].rearrange("(dc p) b -> p dc b", p=128),
                        in_=zf[:])
                    zg3 = zg[:].rearrange("p (c b) -> p c b", c=NGI // 128)
                    nc.gpsimd.dma_gather(
                        out_ap=zg3, in_ap=zdr_d[:], idxs_ap=gi_sb[:],
                        num_idxs=NGI, num_idxs_reg=NGI, elem_size=BC,
                        single_packet=False)
                # U = z_i * z_j for out (chunks 0..3 * 4..7) and act (8..11 * 12..15)
                nc.vector.tensor_tensor(out=u_t[:, :PC * BC],
                                        in0=zg[:, 0:PC * BC],
                                        in1=zg[:, PC * BC:2 * PC * BC], op=MULT)
                nc.vector.tensor_tensor(out=u_t[:, PC * BC:],
                                        in0=zg[:, 2 * PC * BC:3 * PC * BC],
                                        in1=zg[:, 3 * PC * BC:], op=MULT)
                # S = S * r + U
                if with_r:
                    rr3 = rr_sb[:].rearrange("p c -> p c")
                    nc.vector.tensor_tensor(
                        out=s_t[:].rearrange("p (c b) -> p c b", c=2 * PC),
                        in0=s_t[:].rearrange("p (c b) -> p c b", c=2 * PC),
                        in1=_bcast(rr3, BC), op=MULT)
                nc.vector.tensor_tensor(out=s_t[:], in0=s_t[:], in1=u_t[:], op=ADD)
                nc.vector.tensor_copy(out=s_bf[:], in_=s_t[:])

                # y_t = S_out @ W_out ; q_t = S_act @ W_act
                y_ps = ppo.tile([128, 2 * PC * BC], F32, tag="out")
                for mc in range(PC):
                    for kc in range(PC):
                        nc.tensor.matmul(
                            out=y_ps[:, mc * BC:(mc + 1) * BC],
                            lhsT=wo_sb[:, kc * DOUT + mc * 128: kc * DOUT + (mc + 1) * 128],
                            rhs=s_bf[:, kc * BC:(kc + 1) * BC],
                            start=(kc == 0), stop=(kc == PC - 1))
                for mc in range(PC):
                    for kc in range(PC):
                        nc.tensor.matmul(
                            out=y_ps[:, (PC + mc) * BC:(PC + mc + 1) * BC],
                            lhsT=wa_sb[:, kc * DACT + mc * 128: kc * DACT + (mc + 1) * 128],
                            rhs=s_bf[:, PC * BC + kc * BC: PC * BC + (kc + 1) * BC],
                            start=(kc == 0), stop=(kc == PC - 1))
                y_sb = wp.tile([128, 2 * PC * BC], F32, tag="ysb")
                nc.scalar.copy(out=y_sb[:], in_=y_ps[:])
                nc.sync.dma_start(
                    out=y_d[t - 1].rearrange("(mc p) b -> p mc b", p=128),
                    in_=y_sb[:, :PC * BC])
                nc.sync.dma_start(
                    out=q_d[t - 1].rearrange("(mc p) b -> p mc b", p=128),
                    in_=y_sb[:, PC * BC:])

            if DBG_DUMP:
                nc.gpsimd.dma_start(out=dbg_feats[:], in_=featsT[:])
                nc.sync.dma_start(out=dbg_ct[:], in_=cT[:])
                nc.gpsimd.dma_start(out=dbg_h1[:], in_=hist[:, 0:DC * BC])
                nc.sync.dma_start(out=dbg_zf[:], in_=zf[:])
                nc.sync.dma_start(out=dbg_zg[:], in_=zg[:])
                nc.sync.dma_start(out=dbg_s[:], in_=s_t[:])
                nc.gpsimd.dma_start(out=dbg_acc[:], in_=relu_r[:])

    nc.compile()
    return nc


def kernel(x, W_in, b_in, z0, W_syn, b_syn, nlm_w1, nlm_b1, nlm_w2, nlm_b2,
           decay_out, decay_action, W_out, b_out, W_act, b_act,
           pairs_out, pairs_action):
    x = np.asarray(x); W_in = np.asarray(W_in); b_in = np.asarray(b_in)
    z0 = np.asarray(z0); W_syn = np.asarray(W_syn); b_syn = np.asarray(b_syn)
    nlm_w1 = np.asarray(nlm_w1); nlm_b1 = np.asarray(nlm_b1)
    nlm_w2 = np.asarray(nlm_w2); nlm_b2 = np.asarray(nlm_b2)
    decay_out = np.asarray(decay_out); decay_action = np.asarray(decay_action)
    W_out = np.asarray(W_out); b_out = np.asarray(b_out)
    W_act = np.asarray(W_act); b_act = np.asarray(b_act)
    pairs_out = np.asarray(pairs_out); pairs_action = np.asarray(pairs_action)

    Wu = W_syn[:D]                    # z part
    Wl = W_syn[D:]                    # feats part
    r_out = np.exp(-np.abs(decay_out)).astype(np.float32)
    r_act = np.exp(-np.abs(decay_action)).astype(np.float32)

    with_b1 = bool(np.any(nlm_b1))
    with_b2 = bool(np.any(nlm_b2))
    with_r = not bool(np.allclose(r_out, 1.0) and np.allclose(r_act, 1.0))

    key = (with_b1, with_b2, with_r)
    if key not in _cache:
        _cache[key] = _build(*key)
    nc = _cache[key]

    def to_pd(v):     # [D] -> [dp, dc] with d = dc*128 + dp
        return np.ascontiguousarray(v.reshape(DC, 128).T)

    bf = lambda a: a.astype(ml_dtypes.bfloat16)

    # gather index list: chunk c = q*PC + pc holds rows for p = pc*128 + pp
    # ZG[pp, c, :] = z_dram[idx_flat[c*128 + pp]]
    idx_flat = np.concatenate([
        pairs_out[:, 0], pairs_out[:, 1],
        pairs_action[:, 0], pairs_action[:, 1]]).astype(np.int64)
    gi = np.zeros((128, NGI // 16), np.int16)
    for g in range(8):
        gi[16 * g + (np.arange(NGI) % 16), np.arange(NGI) // 16] = idx_flat
    def to_ppc(v):    # [P] -> [pp, pc] with p = pc*128 + pp
        return np.ascontiguousarray(v.reshape(PC, 128).T)

    cb = (b_in @ Wl + b_syn).astype(np.float32)       # [D]
    wn = np.zeros((128, H * 8 * DC), np.float32)      # [dp, (h, u, dc)]
    for h in range(H):
        for u in range(8):
            wn[:, (h * 8 + u) * DC:(h * 8 + u + 1) * DC] = to_pd(nlm_w1[M - 1 - u, h, :])
    w2p = np.zeros((128, H * DC), np.float32)
    b1p = np.zeros((128, H * DC), np.float32)
    for h in range(H):
        w2p[:, h * DC:(h + 1) * DC] = to_pd(nlm_w2[h, :])
        b1p[:, h * DC:(h + 1) * DC] = to_pd(nlm_b1[0, h, :])
    rr = np.concatenate([to_ppc(r_out), to_ppc(r_act)], axis=1)  # [128, 2*PC]

    # column-block Wl: [mc*D + k, m] = Wl[k, mc*128+m]
    wl_blocked = np.ascontiguousarray(
        Wl.reshape(D, DC, 128).transpose(1, 0, 2).reshape(DC * D, 128))
    common = {
        "win": bf(W_in), "wl": bf(wl_blocked), "wu": bf(Wu),
        "cb": to_pd(cb), "z0t": to_pd(z0.astype(np.float32)),
        "wn": bf(wn), "w2p": bf(w2p), "b1p": b1p,
        "b2p": to_pd(nlm_b2[0].astype(np.float32)),
        "rr": rr.astype(np.float32), "gi": gi,
        "wo": bf(W_out), "wa": bf(W_act),
    }
    in_maps = []
    for c in range(NCORES):
        xs = x[c * BC:(c + 1) * BC]                   # [BC, DIN]
        in_maps.append({**common, "xT": bf(np.ascontiguousarray(xs.T))})

    trace = os.environ.get("DBG_TRACE", "0") == "1"
    res = run_bass_kernel_spmd(nc, in_maps[:DBG_CORES], core_ids=list(range(DBG_CORES)),
                               trace=trace)
    if trace and res.exec_time_ns is not None:
        print(f"HW exec time: {res.exec_time_ns} ns")
    global _last_res
    _last_res = res

    ys = np.zeros((B, T, DOUT), np.float32)
    qs = np.zeros((B, T, DACT), np.float32)
    for c in range(DBG_CORES):
        ys[c * BC:(c + 1) * BC] = res.results[c]["y"].transpose(2, 0, 1)
        qs[c * BC:(c + 1) * BC] = res.results[c]["q"].transpose(2, 0, 1)
    ys += b_out[None, None, :]
    qs += b_act[None, None, :]
    return ys, qs



# revision 21
# speedup vs baseline: 2.0502x; 2.0502x over previous
"""Trainium2 Bass kernel for the ContinuousThoughtMachine problem.

Strategy: data-parallel over batch B=512 across 8 NeuronCores (64 per core).
T-layout on device: feature dims on partitions, batch on the free dim.

v2 design (vs v1 baseline):
  - synapse matmul in fp8-e4m3 (weights x64, activations x8; scales folded
    into host-preprocessed NLM weights / W_out) -> FWL 4x weight load
  - cT prefolded on host: cT = x @ (W_in @ W_syn[D:]) + cb, killing the
    256-MM feats@Wl chain and the 4MB Wl stream
  - NLM weights pre-broadcast over batch on host (wnx/w2x) so every DVE
    tensor_tensor is contiguous bf16 -> 2x perf mode (267ns vs 3000ns)
  - z gather: descriptors prepared ONCE per block (gpsimd SWDGE prep),
    fired with trigger_dma after the block's z writes land; gathers are
    blocked over ticks (6+2) so only 2 preps exist
  - sync path (U=z_i*z_j, S accumulate) on gpsimd, freeing DVE
  - tiny keepalive matmuls after each relu keep the PE HAM-warm so fp8
    streams at 2.4GHz
"""
import sys
sys.path.insert(0, '/opt/trn_rl_repo')
import os
import numpy as np
import ml_dtypes

import concourse.bass as bass
import concourse.mybir as mybir
from concourse import bacc
from concourse.bass_utils import run_bass_kernel_spmd
from concourse.tile import TileContext
from concourse.tile_sem_assignment import PROC_NAME_TO_IDX

# problem dims
B, T, D, DIN, M, H, P, DOUT, DACT = 512, 8, 2048, 512, 16, 4, 512, 512, 512
NCORES = 8
BC = B // NCORES          # 64 batch per core
DC = D // 128             # 16 d-chunks
PC = P // 128             # 4 p-chunks
KIN = DIN // 128          # 4 input chunks
NGI = 4 * P               # 2048 gather indices (i_out, j_out, i_act, j_act)
K = DC * BC               # 1024 free elems per d-slab per partition

SW = 64.0                 # fp8 scale for Wu
SZ = 8.0                  # fp8 scale for z
SWZ = SW * SZ

BLOCKS = ((1, 2, 3, 4, 5, 6), (7, 8))   # gather tick blocks

F32 = mybir.dt.float32
BF16 = mybir.dt.bfloat16
FP8 = mybir.dt.float8e4
I16 = mybir.dt.int16
MULT = mybir.AluOpType.mult
ADD = mybir.AluOpType.add

_cache = {}

DBG_TICKS = int(os.environ.get("DBG_TICKS", "8"))
DBG_CORES = int(os.environ.get("DBG_CORES", str(NCORES)))
KEEPALIVE = os.environ.get("KEEPALIVE", "1") == "1"
GATHER_MODE = os.environ.get("GATHER_MODE", "prep")      # prep | direct
SINGLE_PACKET = os.environ.get("SINGLE_PACKET", "1") == "1"


def _build(with_b1, with_b2, with_r, with_cb):
    nc = bacc.Bacc("TRN2", target_bir_lowering=False, debug=False,
                   detect_race_conditions=os.environ.get("DBG_NORACE", "0") != "1")

    ticks = list(range(1, DBG_TICKS + 1))
    blocks = [tuple(t for t in blk if t <= DBG_TICKS) for blk in BLOCKS]
    blocks = [b for b in blocks if b]

    # ---------------- DRAM tensors ----------------
    xT_d = nc.dram_tensor("xT", [DIN, BC], BF16, kind="ExternalInput")
    winl_d = nc.dram_tensor("winl", [DIN, D], BF16, kind="ExternalInput")   # SWZ*(W_in@Wl) lhsT
    cbx_d = nc.dram_tensor("cbx", [128, K], F32, kind="ExternalInput")      # SWZ*(b_in@Wl+b_syn) bcast
    wu8_d = nc.dram_tensor("wu8", [D, D], FP8, kind="ExternalInput")        # SW*Wu lhsT
    z0x_d = nc.dram_tensor("z0x", [128, K], FP8, kind="ExternalInput")      # SZ*z0 bcast
    wnx_d = nc.dram_tensor("wnx", [128, H * 8 * K], BF16, kind="ExternalInput")  # w1/SWZ [(h,j,dc,b)]
    w2x_d = nc.dram_tensor("w2x", [128, H * K], BF16, kind="ExternalInput")      # SZ*w2 [(h,dc,b)]
    b1x_d = nc.dram_tensor("b1x", [128, H * K], BF16, kind="ExternalInput")
    b2x_d = nc.dram_tensor("b2x", [128, K], BF16, kind="ExternalInput")
    rrx_d = nc.dram_tensor("rrx", [128, 2 * PC * BC], F32, kind="ExternalInput")
    gi_d = nc.dram_tensor("gi", [128, NGI // 16], I16, kind="ExternalInput")
    wo_d = nc.dram_tensor("wo", [P, DOUT], BF16, kind="ExternalInput")      # (W_out/SZ^2) lhsT
    wa_d = nc.dram_tensor("wa", [P, DACT], BF16, kind="ExternalInput")

    zdr_d = [nc.dram_tensor(f"zdr{i}", [D, len(blk) * BC], BF16, kind="Internal")
             for i, blk in enumerate(blocks)]
    y_d = nc.dram_tensor("y", [T, DOUT, BC], F32, kind="ExternalOutput")
    q_d = nc.dram_tensor("q", [T, DACT, BC], F32, kind="ExternalOutput")

    with TileContext(nc) as tc:
        with tc.tile_pool(name="const", bufs=1) as cp, \
             tc.tile_pool(name="work", bufs=2) as wp, \
             tc.tile_pool(name="psum", bufs=2, space="PSUM") as pp, \
             tc.tile_pool(name="psum_o", bufs=2, space="PSUM") as ppo, \
             tc.tile_pool(name="psum_k", bufs=1, space="PSUM") as ppk:

            # ---------------- resident loads ----------------
            xt_sb = cp.tile([128, KIN * BC], BF16)
            nc.sync.dma_start(out=xt_sb[:],
                              in_=xT_d[:].rearrange("(kc p) b -> p kc b", p=128))
            winl_sb = cp.tile([128, KIN * D], BF16)
            for kc in range(KIN):
                nc.sync.dma_start(out=winl_sb[:, kc * D:(kc + 1) * D],
                                  in_=winl_d[kc * 128:(kc + 1) * 128, :])
            zq = cp.tile([128, K], FP8)
            nc.sync.dma_start(out=zq[:], in_=z0x_d[:])
            wu8_sb = cp.tile([128, DC * D], FP8)        # [dp, (kc, m)]
            for kc in range(DC):
                nc.sync.dma_start(out=wu8_sb[:, kc * D:(kc + 1) * D],
                                  in_=wu8_d[kc * 128:(kc + 1) * 128, :])
            cbx_sb = None
            if with_cb:
                cbx_sb = cp.tile([128, K], F32)
                nc.sync.dma_start(out=cbx_sb[:], in_=cbx_d[:])
            wnx_sb = cp.tile([128, H * 8 * K], BF16)
            # j-descending per h so early ticks' slabs land first
            for j in range(7, -1, -1):
                for h in range(H):
                    off = (h * 8 + j) * K
                    nc.sync.dma_start(out=wnx_sb[:, off:off + K],
                                      in_=wnx_d[:, off:off + K])
            w2x_sb = cp.tile([128, H * K], BF16)
            nc.sync.dma_start(out=w2x_sb[:], in_=w2x_d[:])
            b1x_sb = b2x_sb = rrx_sb = None
            if with_b1:
                b1x_sb = cp.tile([128, H * K], BF16)
                nc.sync.dma_start(out=b1x_sb[:], in_=b1x_d[:])
            if with_b2:
                b2x_sb = cp.tile([128, K], BF16)
                nc.sync.dma_start(out=b2x_sb[:], in_=b2x_d[:])
            if with_r:
                rrx_sb = cp.tile([128, 2 * PC * BC], F32)
                nc.sync.dma_start(out=rrx_sb[:], in_=rrx_d[:])
            gi_sb = cp.tile([128, NGI // 16], I16)
            nc.sync.dma_start(out=gi_sb[:], in_=gi_d[:])
            wo_sb = cp.tile([128, PC * DOUT], BF16)
            for kc in range(PC):
                nc.sync.dma_start(out=wo_sb[:, kc * DOUT:(kc + 1) * DOUT],
                                  in_=wo_d[kc * 128:(kc + 1) * 128, :])
            wa_sb = cp.tile([128, PC * DACT], BF16)
            for kc in range(PC):
                nc.sync.dma_start(out=wa_sb[:, kc * DACT:(kc + 1) * DACT],
                                  in_=wa_d[kc * 128:(kc + 1) * 128, :])

            # ---------------- gather buffers/sems (preps emitted per block) ----
            zg_sb = [cp.tile([128, 16 * len(blk) * BC], BF16, name=f"zg{i}")
                     for i, blk in enumerate(blocks)]

            def emit_gather(i):
                # prep must be emitted AFTER the block's z writes: Tile's
                # program order is emission order, and the deferred RAW edge
                # on zdr lands on the trigger via the prep's recorded reads.
                # sem must be the Tile DMASW lane sem the prep will be
                # assigned (Pool DMAs take lanes round-robin; the preps are
                # the only Pool DMAs here, so prep i -> DMASW{i}).
                kw = {}
                if GATHER_MODE == "prep":
                    kw = dict(prepare_only=True,
                              sem=tc.sems[PROC_NAME_TO_IDX[f"DMASW{i}"]])
                nc.gpsimd.dma_gather(
                    out_ap=zg_sb[i][:].rearrange("p (c e) -> p c e", c=16),
                    in_ap=zdr_d[i][:], idxs_ap=gi_sb[:],
                    num_idxs=NGI, num_idxs_reg=NGI,
                    elem_size=len(blocks[i]) * BC,
                    single_packet=SINGLE_PACKET, queue_num=0, **kw)
                if GATHER_MODE == "prep":
                    nc.gpsimd.trigger_dma(count=None, queue_num=0)

            # ---------------- cT (once) ----------------
            # cTx[d, b] = SWZ * (x @ (W_in@Wl))[b, d]  (+ cbx)
            c_ps = pp.tile([128, K], F32, tag="syn")
            for mc in range(DC):
                for kc in range(KIN):
                    nc.tensor.matmul(
                        out=c_ps[:, mc * BC:(mc + 1) * BC],
                        lhsT=winl_sb[:, kc * D + mc * 128: kc * D + (mc + 1) * 128],
                        rhs=xt_sb[:, kc * BC:(kc + 1) * BC],
                        start=(kc == 0), stop=(kc == KIN - 1))
            cTx = cp.tile([128, K], F32)
            if with_cb:
                nc.vector.tensor_tensor(out=cTx[:], in0=c_ps[:], in1=cbx_sb[:], op=ADD)
            else:
                nc.scalar.copy(out=cTx[:], in_=c_ps[:])

            # ---------------- state buffers ----------------
            hist = cp.tile([128, T * K], BF16)          # slab s-1 = 512*pre_s
            acc = cp.tile([128, H * K], BF16)
            tmp = cp.tile([128, K], BF16)               # shared scratch slab
            relu_r = cp.tile([128, H * K], BF16)
            zfb = cp.tile([128, K], BF16)               # SZ * z_t
            u_t = cp.tile([128, 2 * PC * BC], F32)
            s_t = cp.tile([128, 2 * PC * BC], F32)
            nc.vector.memset(s_t[:], 0.0)
            ka_ps = ppk.tile([128, 16], F32, tag="ka")

            wnx3 = wnx_sb[:].rearrange("p (h j e) -> p (h j) e", h=H, j=8)

            def emit_block_tail(bi):
                """U/S/y for every tick in blocks[bi]; call after its trigger."""
                blk = blocks[bi]
                nblk = len(blk)
                zg3 = zg_sb[bi][:].rearrange("p (c tb b) -> p c tb b", c=16, tb=nblk)
                for tb, t in enumerate(blk):
                    u3 = u_t[:].rearrange("p (g c b) -> p (g c) b", g=2, c=PC)
                    nc.gpsimd.tensor_tensor(
                        out=u3[:, 0:PC, :],
                        in0=zg3[:, 0:PC, tb, :], in1=zg3[:, PC:2 * PC, tb, :], op=MULT)
                    nc.gpsimd.tensor_tensor(
                        out=u3[:, PC:2 * PC, :],
                        in0=zg3[:, 2 * PC:3 * PC, tb, :], in1=zg3[:, 3 * PC:, tb, :],
                        op=MULT)
                    if with_r:
                        nc.gpsimd.tensor_tensor(out=s_t[:], in0=s_t[:], in1=rrx_sb[:],
                                                op=MULT)
                    nc.gpsimd.tensor_tensor(out=s_t[:], in0=s_t[:], in1=u_t[:], op=ADD)
                    s_bf = wp.tile([128, 2 * PC * BC], BF16, tag="sbf")
                    nc.gpsimd.tensor_copy(out=s_bf[:], in_=s_t[:])

                    y_ps = ppo.tile([128, 2 * PC * BC], F32, tag="out")
                    for mc in range(PC):
                        for kc in range(PC):
                            nc.tensor.matmul(
                                out=y_ps[:, mc * BC:(mc + 1) * BC],
                                lhsT=wo_sb[:, kc * DOUT + mc * 128: kc * DOUT + (mc + 1) * 128],
                                rhs=s_bf[:, kc * BC:(kc + 1) * BC],
                                start=(kc == 0), stop=(kc == PC - 1))
                    for mc in range(PC):
                        for kc in range(PC):
                            nc.tensor.matmul(
                                out=y_ps[:, (PC + mc) * BC:(PC + mc + 1) * BC],
                                lhsT=wa_sb[:, kc * DACT + mc * 128: kc * DACT + (mc + 1) * 128],
                                rhs=s_bf[:, PC * BC + kc * BC: PC * BC + (kc + 1) * BC],
                                start=(kc == 0), stop=(kc == PC - 1))
                    y_sb = wp.tile([128, 2 * PC * BC], F32, tag="ysb")
                    nc.scalar.copy(out=y_sb[:], in_=y_ps[:])
                    nc.sync.dma_start(
                        out=y_d[t - 1].rearrange("(mc p) b -> p mc b", p=128),
                        in_=y_sb[:, :PC * BC])
                    nc.sync.dma_start(
                        out=q_d[t - 1].rearrange("(mc p) b -> p mc b", p=128),
                        in_=y_sb[:, PC * BC:])

            # ---------------- tick loop ----------------
            for t in ticks:
                bi, tb = next((i, blk.index(t)) for i, blk in enumerate(blocks)
                              if t in blk)

                # --- synapse: hist_t = SWZ*pre = wu8.T @ zq (+cTx) ---
                syn_ps = pp.tile([128, K], F32, tag="syn")
                for mc in range(DC):
                    for kc in range(DC):
                        nc.tensor.matmul(
                            out=syn_ps[:, mc * BC:(mc + 1) * BC],
                            lhsT=wu8_sb[:, kc * D + mc * 128: kc * D + (mc + 1) * 128],
                            rhs=zq[:, kc * BC:(kc + 1) * BC],
                            start=(kc == 0), stop=(kc == DC - 1))

                # --- NLM catchup: s < t terms (overlap with synapse) ---
                # slab for tick t, step s: hist slab s-1 x wnx[h, j=8-t+s-1]
                for h in range(H):
                    a_h = acc[:, h * K:(h + 1) * K]
                    m_h = tmp[:, 0:K]
                    for s in range(1, t):
                        w_in1 = wnx3[:, h * 8 + (8 - t) + (s - 1), :]
                        hs = hist[:, (s - 1) * K: s * K]
                        if s == 1:
                            nc.vector.tensor_tensor(out=a_h, in0=hs, in1=w_in1, op=MULT)
                        else:
                            nc.vector.tensor_tensor(out=m_h, in0=hs, in1=w_in1, op=MULT)
                            nc.vector.tensor_tensor(out=a_h, in0=a_h, in1=m_h, op=ADD)

                # --- hist slab write (critical) ---
                hslot = hist[:, (t - 1) * K: t * K]
                nc.vector.tensor_tensor(out=hslot, in0=syn_ps[:], in1=cTx[:], op=ADD)

                # --- NLM critical term + relu ---
                for h in range(H):
                    a_h = acc[:, h * K:(h + 1) * K]
                    m_h = tmp[:, 0:K]
                    w_in1 = wnx3[:, h * 8 + 7, :]
                    if t == 1:
                        nc.vector.tensor_tensor(out=a_h, in0=hslot, in1=w_in1, op=MULT)
                    else:
                        nc.vector.tensor_tensor(out=m_h, in0=hslot, in1=w_in1, op=MULT)
                        nc.vector.tensor_tensor(out=a_h, in0=a_h, in1=m_h, op=ADD)
                    if with_b1:
                        nc.vector.tensor_tensor(out=a_h, in0=a_h,
                                                in1=b1x_sb[:, h * K:(h + 1) * K], op=ADD)
                    nc.scalar.activation(
                        out=relu_r[:, h * K:(h + 1) * K], in_=a_h,
                        func=mybir.ActivationFunctionType.Relu)
                    if KEEPALIVE:
                        nc.tensor.matmul(out=ka_ps[:, 0:8],
                                         lhsT=w2x_sb[:, 0:128],
                                         rhs=relu_r[:, h * K: h * K + 8],
                                         start=True, stop=True)

                # --- z combine: zfb = SZ*z = sum_h w2x_h * relu_h ---
                def r_h(h):
                    return relu_r[:, h * K:(h + 1) * K]

                def w2_h(h):
                    return w2x_sb[:, h * K:(h + 1) * K]

                nc.vector.tensor_tensor(out=zfb[:], in0=r_h(0), in1=w2_h(0), op=MULT)
                m_z = tmp[:, 0:K]
                for h in range(1, H):
                    nc.vector.tensor_tensor(out=m_z, in0=r_h(h), in1=w2_h(h), op=MULT)
                    nc.vector.tensor_tensor(out=zfb[:], in0=zfb[:], in1=m_z, op=ADD)
                if with_b2:
                    nc.vector.tensor_tensor(out=zfb[:], in0=zfb[:], in1=b2x_sb[:], op=ADD)

                # --- next-tick rhs (fp8) ---
                nc.vector.tensor_copy(out=zq[:], in_=zfb[:])

                # --- z -> DRAM block column ---
                nc.sync.dma_start(
                    out=zdr_d[bi][:, tb * BC:(tb + 1) * BC].rearrange(
                        "(dc p) b -> p dc b", p=128),
                    in_=zfb[:])

                # lag block-0 tail by one tick so its out-MMs fill PE gaps
                if len(blocks) > 1 and blocks[0][-1] == t - 1:
                    emit_block_tail(0)

                # --- block boundary: fire prepared gather ---
                if tb == len(blocks[bi]) - 1:
                    emit_gather(bi)
                    if bi == len(blocks) - 1:
                        if len(blocks) > 1 and blocks[0][-1] == t:
                            emit_block_tail(0)       # block 0 not yet drained
                        emit_block_tail(bi)          # final block: drain now

    nc.compile()
    return nc


def kernel(x, W_in, b_in, z0, W_syn, b_syn, nlm_w1, nlm_b1, nlm_w2, nlm_b2,
           decay_out, decay_action, W_out, b_out, W_act, b_act,
           pairs_out, pairs_action):
    x = np.asarray(x); W_in = np.asarray(W_in); b_in = np.asarray(b_in)
    z0 = np.asarray(z0); W_syn = np.asarray(W_syn); b_syn = np.asarray(b_syn)
    nlm_w1 = np.asarray(nlm_w1); nlm_b1 = np.asarray(nlm_b1)
    nlm_w2 = np.asarray(nlm_w2); nlm_b2 = np.asarray(nlm_b2)
    decay_out = np.asarray(decay_out); decay_action = np.asarray(decay_action)
    W_out = np.asarray(W_out); b_out = np.asarray(b_out)
    W_act = np.asarray(W_act); b_act = np.asarray(b_act)
    pairs_out = np.asarray(pairs_out); pairs_action = np.asarray(pairs_action)

    Wu = W_syn[:D].astype(np.float32)
    Wl = W_syn[D:].astype(np.float32)
    r_out = np.exp(-np.abs(decay_out)).astype(np.float32)
    r_act = np.exp(-np.abs(decay_action)).astype(np.float32)
    cb = (b_in @ Wl + b_syn).astype(np.float32)

    with_b1 = bool(np.any(nlm_b1))
    with_b2 = bool(np.any(nlm_b2))
    with_r = not bool(np.allclose(r_out, 1.0) and np.allclose(r_act, 1.0))
    with_cb = bool(np.any(cb))

    key = (with_b1, with_b2, with_r, with_cb)
    if key not in _cache:
        _cache[key] = _build(*key)
    nc = _cache[key]

    bf = lambda a: a.astype(ml_dtypes.bfloat16)
    e4 = lambda a: np.clip(a, -240, 240).astype(ml_dtypes.float8_e4m3fn)

    def to_pd(v):     # [D] -> [dp, dc] with d = dc*128 + dp
        return np.ascontiguousarray(v.reshape(DC, 128).T)

    def expand(v):    # [D] -> [128, DC*BC] broadcast over b
        return np.ascontiguousarray(
            np.broadcast_to(to_pd(v)[:, :, None], (128, DC, BC)).reshape(128, DC * BC))

    # gather index list: chunk c holds rows for p = c*128 + pp
    idx_flat = np.concatenate([
        pairs_out[:, 0], pairs_out[:, 1],
        pairs_action[:, 0], pairs_action[:, 1]]).astype(np.int64)
    gi = np.zeros((128, NGI // 16), np.int16)
    for g in range(8):
        gi[16 * g + (np.arange(NGI) % 16), np.arange(NGI) // 16] = idx_flat

    def to_ppc_expand(v):  # [P] -> [128, PC*BC]
        pv = np.ascontiguousarray(v.reshape(PC, 128).T)
        return np.ascontiguousarray(
            np.broadcast_to(pv[:, :, None], (128, PC, BC)).reshape(128, PC * BC))

    # NLM w1: wnx[h, j] = w1[M-1-(7-j), h, :] / SWZ, broadcast over b
    wnx = np.zeros((128, H * 8 * K), np.float32)
    for h in range(H):
        for j in range(8):
            u = 7 - j
            wnx[:, (h * 8 + j) * K:(h * 8 + j + 1) * K] = \
                expand(nlm_w1[M - 1 - u, h, :] / SWZ)
    w2x = np.zeros((128, H * K), np.float32)
    b1x = np.zeros((128, H * K), np.float32)
    for h in range(H):
        w2x[:, h * K:(h + 1) * K] = expand(nlm_w2[h, :] * SZ)
        b1x[:, h * K:(h + 1) * K] = expand(nlm_b1[0, h, :])
    rrx = np.concatenate([to_ppc_expand(r_out), to_ppc_expand(r_act)], axis=1)

    winl = (SWZ * (W_in.astype(np.float32) @ Wl))

    common = {
        "winl": bf(winl), "cbx": expand(SWZ * cb),
        "wu8": e4(Wu * SW), "z0x": e4(expand(z0.astype(np.float32) * SZ)),
        "wnx": bf(wnx), "w2x": bf(w2x),
        "b1x": bf(b1x), "b2x": bf(expand(nlm_b2[0].astype(np.float32) * SZ)),
        "rrx": rrx.astype(np.float32), "gi": gi,
        "wo": bf(W_out / (SZ * SZ)), "wa": bf(W_act / (SZ * SZ)),
    }
    in_maps = []
    for c in range(DBG_CORES):
        xs = x[c * BC:(c + 1) * BC]                   # [BC, DIN]
        in_maps.append({**common, "xT": bf(np.ascontiguousarray(xs.T))})

    trace = os.environ.get("DBG_TRACE", "0") == "1"
    res = run_bass_kernel_spmd(nc, in_maps, core_ids=list(range(DBG_CORES)),
                               trace=trace)
    if trace and res.exec_time_ns is not None:
        print(f"HW exec time: {res.exec_time_ns} ns")
    global _last_res
    _last_res = res

    ys = np.zeros((B, T, DOUT), np.float32)
    qs = np.zeros((B, T, DACT), np.float32)
    for c in range(DBG_CORES):
        ys[c * BC:(c + 1) * BC] = res.results[c]["y"].transpose(2, 0, 1)
        qs[c * BC:(c + 1) * BC] = res.results[c]["q"].transpose(2, 0, 1)
    ys += b_out[None, None, :]
    qs += b_act[None, None, :]
    return ys, qs
